# revision 1
# baseline (speedup 1.0000x reference)
"""Trainium2 Bass kernel for nn_Experts (topk_masking).

Math (reference):
  R = concat(h,us,ue) @ W_r.T + b_r                       [1,1,512]
  x = concat(u, R.broadcast)                              [1,S,1536]
  h1 = (x @ W_nn.T + b_nn).reshape(S,512,16)
  h2 = (x @ W_no.T + b_no).reshape(S,512,16) * noise
  g  = top2-masked softmax over experts of (h1+h2)
  e  = (x @ W_E.T + b_E).reshape(S,512,16)
  out = (g*e).mean(-1)                                    [1,S,512]

Sharding: the NE*DIM output-feature dim of the three projections is sharded
across 8 cores (64 dims x 16 experts each, contiguous feature slice). The
token-independent R-path is folded into a per-feature constant c[f] computed
once per core, so the per-token matmuls contract only over u's 1024 features.

Precision: gating matmuls use a 2-term fp32 split (11 explicit mantissa bits
+ residual) through the PE's float32r mode (verified: <=11-bit operands pass
through exactly), plus a bf16 cross-term; this lands the gating logits at
~fp32 accuracy so top-2 selection matches the fp32 reference. The e-matmul
runs in bf16 (smooth, no selection discontinuity).
"""
import numpy as np
import ml_dtypes

DIM = 512
NE = 16
S = 4096
KU = 2 * DIM        # u features = 1024
KR = DIM            # R features = 512
KX = 5 * DIM        # concat(h,us,ue) = 2560
NCORES = 8
DL = DIM // NCORES  # 64 dims per core
FL = DL * NE        # 1024 features per core
MCH = S // 128      # 32 token chunks

_MASK11 = np.uint32(0xFFFFF000)  # keep 11 explicit mantissa bits

TRACE = False
DEBUG = False
_CACHE = {}


def _trunc11(a):
    a = np.ascontiguousarray(a, dtype=np.float32)
    return (a.view(np.uint32) & _MASK11).view(np.float32)


def _build():
    import concourse.bass as bass
    import concourse.mybir as mybir
    import concourse.tile as tile
    from concourse import bacc
    from contextlib import ExitStack

    F32 = mybir.dt.float32
    F32R = mybir.dt.float32r
    BF16 = mybir.dt.bfloat16
    U32 = mybir.dt.uint32
    AX = mybir.AxisListType
    OP = mybir.AluOpType
    ACTF = mybir.ActivationFunctionType

    nc = bacc.Bacc("TRN2", target_bir_lowering=False, debug=False,
                   num_devices=NCORES)

    def dram(name, shape, dt, kind="ExternalInput"):
        return nc.dram_tensor(name, shape, dt, kind=kind)

    # per-core inputs (same names on every core; data differs per core)
    uhT = dram("uhT", [KU, S], F32R)
    ulT = dram("ulT", [KU, S], F32R)
    u8T = dram("u8T", [KU, S], BF16)
    whnnT = dram("whnnT", [KU, FL], F32R)
    whnoT = dram("whnoT", [KU, FL], F32R)
    wl8T = dram("wl8T", [KU, 2 * FL], BF16)   # [:, :FL]=nn resid, [:, FL:]=no resid
    we8T = dram("we8T", [KU, FL], BF16)
    noise_c = dram("noise_c", [S, FL], F32)
    hxf = dram("hxf", [KX], F32)
    wrT = dram("wrT", [KX, KR], F32)
    b_r = dram("b_r", [KR], F32)
    wRh_nn = dram("wRh_nn", [KR, FL], F32R)
    wRl_nn = dram("wRl_nn", [KR, FL], F32R)
    wRh_no = dram("wRh_no", [KR, FL], F32R)
    wRl_no = dram("wRl_no", [KR, FL], F32R)
    wR_E = dram("wR_E", [KR, FL], F32R)
    bias_c = dram("bias_c", [3 * FL], F32)
    out_c = dram("out_c", [S, DL], F32, kind="ExternalOutput")
    dbg = {}
    if DEBUG:
        for nm in ["h1", "h2", "e", "m", "q", "mask"]:
            dbg[nm] = dram("dbg_" + nm, [128, FL], F32, kind="ExternalOutput")
        for nm in ["v1", "v2", "s"]:
            dbg[nm] = dram("dbg_" + nm, [128, DL], F32, kind="ExternalOutput")
        dbg["cc"] = dram("dbg_cc", [2, 3 * FL], F32, kind="ExternalOutput")
        dbg["R"] = dram("dbg_R", [128, 4], F32, kind="ExternalOutput")

    with tile.TileContext(nc) as tc, ExitStack() as ctx:
        wpool = ctx.enter_context(tc.tile_pool(name="w", bufs=1))

        # resident weights (one big DMA each)
        whnn_t = wpool.tile([128, 8, FL], F32R)
        whno_t = wpool.tile([128, 8, FL], F32R)
        wl8_t = wpool.tile([128, 8, 2 * FL], BF16)
        we8_t = wpool.tile([128, 8, FL], BF16)
        nc.sync.dma_start(whnn_t[:], whnnT.ap().rearrange("(kc p) f -> p kc f", p=128))
        nc.sync.dma_start(whno_t[:], whnoT.ap().rearrange("(kc p) f -> p kc f", p=128))
        nc.sync.dma_start(wl8_t[:], wl8T.ap().rearrange("(kc p) f -> p kc f", p=128))
        nc.sync.dma_start(we8_t[:], we8T.ap().rearrange("(kc p) f -> p kc f", p=128))

        # survives the whole kernel: bias/R constant rows + ones for the K=2 matmul
        ccsb = wpool.tile([2, 3 * FL], F32R)
        onesf = wpool.tile([2, 128], F32)
        nc.vector.memset(onesf[:], 1.0)
        ones2 = wpool.tile([2, 128], F32R)
        nc.vector.tensor_copy(ones2[:], onesf[:])

        # ---------------- stage 0: R then c ----------------
        with ExitStack() as s0:
            s0sb = s0.enter_context(tc.tile_pool(name="s0sb", bufs=1))
            s0rot = s0.enter_context(tc.tile_pool(name="s0rot", bufs=4))
            s0ps = s0.enter_context(tc.tile_pool(name="s0ps", bufs=1, space="PSUM"))

            hx_t = s0sb.tile([128, 20], F32)
            nc.sync.dma_start(hx_t[:], hxf.ap().rearrange("(kc p) -> p kc", p=128))

            # R = hx @ W_r.T with W_r stationary: out lands as [128, 4]
            # across partitions directly (R[mo*128+p] = psR[p, mo]).
            # NOTE: start=True clears has_written for the whole PSUM bank, so
            # each mo's accumulation chain needs its own bank.
            psR = []
            for mo in range(4):
                psR_mo = s0ps.tile([128, 1], F32, tag=f"psR{mo}")
                psR.append(psR_mo)
            for kc in range(20):
                ksl = slice(kc * 128, (kc + 1) * 128)
                wr_ch = s0rot.tile([128, KR], F32, tag="rotf")
                nc.sync.dma_start(wr_ch[:], wrT.ap()[ksl, :])
                for mo in range(4):
                    msl = slice(mo * 128, (mo + 1) * 128)
                    nc.tensor.matmul(psR[mo][:], wr_ch[:, msl],
                                     hx_t[:, kc:kc + 1],
                                     start=(kc == 0), stop=(kc == 19))

            brt = s0sb.tile([128, 4], F32)
            nc.sync.dma_start(brt[:], b_r.ap().rearrange("(mo p) -> p mo", p=128))
            Rcol = s0sb.tile([128, 4], F32)
            for mo in range(4):
                nc.vector.tensor_add(Rcol[:, mo:mo + 1], psR[mo][:],
                                     brt[:, mo:mo + 1])

            Rh = s0sb.tile([128, 4], F32)
            nc.vector.tensor_scalar(Rh[:].bitcast(U32), Rcol[:].bitcast(U32),
                                    int(_MASK11), None, OP.bitwise_and)
            Rl = s0sb.tile([128, 4], F32)
            nc.vector.tensor_sub(Rl[:], Rcol[:], Rh[:])
            # broadcast along the stationary M dim (value replicated per token)
            Rbch = s0sb.tile([128, 4, 128], F32R)
            nc.vector.tensor_copy(Rbch[:], Rh[:].broadcast_to([128, 4, 128]))
            Rbcl = s0sb.tile([128, 4, 128], F32R)
            nc.vector.tensor_copy(Rbcl[:], Rl[:].broadcast_to([128, 4, 128]))
            if DEBUG:
                nc.sync.dma_start(dbg["R"].ap(), Rcol[:])

            # c pieces: piece 0 -> c_nn, 1 -> c_no, 2 -> c_E (each FL wide)
            # biasb2 doubles as the c+bias staging buffer (updated in place);
            # all DVE work stays on partition 0 (engines need lane-0 alignment)
            biasb2 = s0sb.tile([1, 3 * FL], F32)
            nc.sync.dma_start(biasb2[:],
                              bias_c.ap().rearrange("(o f) -> o f", o=1))
            cpsum = s0ps.tile([128, FL], F32, tag="cps")
            pieces = [(wRh_nn, wRl_nn), (wRh_no, wRl_no), (wR_E, None)]
            for pi, (wh_d, wl_d) in enumerate(pieces):
                for kc in range(4):
                    ksl = slice(kc * 128, (kc + 1) * 128)
                    for half in range(2):
                        fsl = slice(half * 512, (half + 1) * 512)
                        whch = s0rot.tile([128, 512], F32R, tag="rot")
                        nc.sync.dma_start(whch[:], wh_d.ap()[ksl, fsl])
                        nc.tensor.matmul(cpsum[:, fsl], Rbch[:, kc, :], whch[:],
                                         start=(kc == 0), stop=False)
                        nc.tensor.matmul(cpsum[:, fsl], Rbcl[:, kc, :], whch[:],
                                         start=False, stop=False)
                        if wl_d is not None:
                            wlch = s0rot.tile([128, 512], F32R, tag="rot")
                            nc.sync.dma_start(wlch[:], wl_d.ap()[ksl, fsl])
                            nc.tensor.matmul(cpsum[:, fsl], Rbch[:, kc, :],
                                             wlch[:], start=False,
                                             stop=(kc == 3))
                        elif kc == 3:
                            nc.tensor.matmul(cpsum[:, fsl], Rbcl[:, kc, :],
                                             whch[:], start=False, stop=True)
                psl = slice(pi * FL, (pi + 1) * FL)
                nc.vector.tensor_add(biasb2[0:1, psl], cpsum[0:1, :],
                                     biasb2[0:1, psl])

            # split c into 11-bit head + residual, round both to f32r on
            # partition 0, then DMA into the two rows of ccsb
            cht = s0sb.tile([1, 3 * FL], F32)
            nc.vector.tensor_scalar(cht[0:1, :].bitcast(U32),
                                    biasb2[0:1, :].bitcast(U32),
                                    int(_MASK11), None, OP.bitwise_and)
            clt = s0sb.tile([1, 3 * FL], F32)
            nc.vector.tensor_sub(clt[0:1, :], biasb2[0:1, :], cht[0:1, :])
            chr_ = s0sb.tile([1, 3 * FL], F32R)
            nc.vector.tensor_copy(chr_[0:1, :], cht[0:1, :])
            clr_ = s0sb.tile([1, 3 * FL], F32R)
            nc.vector.tensor_copy(clr_[0:1, :], clt[0:1, :])
            nc.sync.dma_start(ccsb[0:1, :], chr_[0:1, :])
            nc.sync.dma_start(ccsb[1:2, :], clr_[0:1, :])
            if DEBUG:
                nc.sync.dma_start(dbg["cc"].ap()[0:1, :], cht[0:1, :])
                nc.sync.dma_start(dbg["cc"].ap()[1:2, :], clt[0:1, :])

        # ---------------- main loop over 32 token chunks ----------------
        spool = ctx.enter_context(tc.tile_pool(name="stream", bufs=2))
        epool = ctx.enter_context(tc.tile_pool(name="epi", bufs=1))
        mpsum = ctx.enter_context(tc.tile_pool(name="mps", bufs=1, space="PSUM"))

        uhT_r = uhT.ap().rearrange("(kc p) t -> p kc t", p=128)
        ulT_r = ulT.ap().rearrange("(kc p) t -> p kc t", p=128)
        u8T_r = u8T.ap().rearrange("(kc p) t -> p kc t", p=128)

        for m in range(MCH):
            tsl = slice(m * 128, (m + 1) * 128)
            xh_t = spool.tile([128, 8, 128], F32R, tag="xh")
            xl_t = spool.tile([128, 8, 128], F32R, tag="xl")
            x8_t = spool.tile([128, 8, 128], BF16, tag="x8")
            nz_t = spool.tile([128, FL], F32, tag="nz")
            nc.sync.dma_start(xh_t[:], uhT_r[:, :, tsl])
            nc.sync.dma_start(xl_t[:], ulT_r[:, :, tsl])
            nc.sync.dma_start(x8_t[:], u8T_r[:, :, tsl])
            nc.sync.dma_start(nz_t[:], noise_c.ap()[tsl, :])

            h1p = mpsum.tile([128, FL], F32, tag="h1")
            h2p = mpsum.tile([128, FL], F32, tag="h2")
            ep = mpsum.tile([128, FL], F32, tag="e")

            for k in range(8):
                lh = xh_t[:, k, :]
                ll = xl_t[:, k, :]
                l8 = x8_t[:, k, :]
                st = (k == 0)
                for half in range(2):
                    fsl = slice(half * 512, (half + 1) * 512)
                    # stationary xh: main gating terms
                    nc.tensor.matmul(h2p[:, fsl], lh, whno_t[:, k, fsl],
                                     start=st, stop=False)
                    nc.tensor.matmul(h1p[:, fsl], lh, whnn_t[:, k, fsl],
                                     start=st, stop=False)
                for half in range(2):
                    fsl = slice(half * 512, (half + 1) * 512)
                    # stationary xl: residual-x terms
                    nc.tensor.matmul(h2p[:, fsl], ll, whno_t[:, k, fsl],
                                     start=False, stop=False)
                    nc.tensor.matmul(h1p[:, fsl], ll, whnn_t[:, k, fsl],
                                     start=False, stop=False)
                for half in range(2):
                    fsl = slice(half * 512, (half + 1) * 512)
                    fsl_no = slice(FL + half * 512, FL + (half + 1) * 512)
                    # stationary x8 (bf16): residual-W cross terms + e matmul
                    nc.tensor.matmul(h2p[:, fsl], l8, wl8_t[:, k, fsl_no],
                                     start=False, stop=False)
                    nc.tensor.matmul(h1p[:, fsl], l8, wl8_t[:, k, fsl],
                                     start=False, stop=False)
                    nc.tensor.matmul(ep[:, fsl], l8, we8_t[:, k, fsl],
                                     start=st, stop=False)

            # bias + R-path constant via K=2 ones-matmul (rows: c_head, c_resid)
            for half in range(2):
                fsl = slice(half * 512, (half + 1) * 512)
                nc.tensor.matmul(h1p[:, fsl], ones2[:], ccsb[:, fsl],
                                 start=False, stop=True)
                nc.tensor.matmul(h2p[:, fsl], ones2[:],
                                 ccsb[:, FL + half * 512:FL + (half + 1) * 512],
                                 start=False, stop=True)
                nc.tensor.matmul(ep[:, fsl], ones2[:],
                                 ccsb[:, 2 * FL + half * 512:2 * FL + (half + 1) * 512],
                                 start=False, stop=True)

            # ---------------- epilogue ----------------
            if DEBUG and m == 0:
                for nm, src in [("h1", h1p), ("h2", h2p), ("e", ep)]:
                    dtmp = epool.tile([128, FL], F32, tag="dbg" + nm)
                    nc.scalar.copy(dtmp[:], src[:])
                    nc.sync.dma_start(dbg[nm].ap(), dtmp[:])
            t_t = epool.tile([128, FL], F32, tag="t")
            nc.vector.tensor_mul(t_t[:], h2p[:], nz_t[:])
            m_t = epool.tile([128, FL], F32, tag="m")
            nc.vector.tensor_add(m_t[:], t_t[:], h1p[:])

            mg = m_t[:].rearrange("p (d e) -> p d e", e=NE)
            v1 = epool.tile([128, DL], F32, tag="v1")
            nc.vector.tensor_reduce(v1[:], mg, AX.X, op=OP.max)
            eq1 = epool.tile([128, FL], F32, tag="eq1")
            nc.vector.tensor_tensor(eq1[:].rearrange("p (d e) -> p d e", e=NE),
                                    mg, v1[:].broadcast_to([128, DL, NE]),
                                    OP.is_equal)
            m2 = epool.tile([128, FL], F32, tag="m2")
            nc.vector.scalar_tensor_tensor(m2[:], eq1[:], -1e30, m_t[:],
                                           OP.mult, OP.add)
            v2 = epool.tile([128, DL], F32, tag="v2")
            nc.vector.tensor_reduce(v2[:], m2[:].rearrange("p (d e) -> p d e", e=NE),
                                    AX.X, op=OP.max)
            mask = epool.tile([128, FL], F32, tag="mask")
            nc.vector.tensor_tensor(mask[:].rearrange("p (d e) -> p d e", e=NE),
                                    mg, v2[:].broadcast_to([128, DL, NE]),
                                    OP.is_ge)
            q = epool.tile([128, FL], F32, tag="q")
            nc.scalar.activation(q[:], m_t[:], ACTF.Exp)

            t1 = epool.tile([128, FL], F32, tag="t1")
            nc.vector.tensor_mul(t1[:], mask[:], ep[:])
            t2 = epool.tile([128, FL], F32, tag="t2")
            nc.vector.tensor_mul(t2[:], t1[:], q[:])
            s_t = epool.tile([128, DL], F32, tag="s")
            nc.vector.tensor_reduce(s_t[:], t2[:].rearrange("p (d e) -> p d e", e=NE),
                                    AX.X, op=OP.add)

            if DEBUG and m == 0:
                for nm, src in [("m", m_t), ("q", q), ("mask", mask)]:
                    nc.sync.dma_start(dbg[nm].ap(), src[:])
                for nm, src in [("v1", v1), ("v2", v2), ("s", s_t)]:
                    nc.sync.dma_start(dbg[nm].ap(), src[:])
            ev12 = epool.tile([128, 2 * DL], F32, tag="ev12")
            nc.scalar.activation(ev12[:, :DL], v1[:], ACTF.Exp)
            nc.scalar.activation(ev12[:, DL:], v2[:], ACTF.Exp)
            z_t = epool.tile([128, DL], F32, tag="z")
            nc.vector.tensor_add(z_t[:], ev12[:, :DL], ev12[:, DL:])
            r_t = epool.tile([128, DL], F32, tag="r")
            nc.vector.reciprocal(r_t[:], z_t[:])
            o_t = epool.tile([128, DL], F32, tag="o")
            nc.vector.scalar_tensor_tensor(o_t[:], s_t[:], 1.0 / NE, r_t[:],
                                           OP.mult, OP.mult)
            nc.sync.dma_start(out_c.ap()[tsl, :], o_t[:])

    nc.compile()
    return nc


def _get_program():
    if "nc" not in _CACHE:
        _CACHE["nc"] = _build()
    return _CACHE["nc"]


def kernel(h, us, ue, u, noise, W_nn, b_nn, W_no, b_no, W_E, b_E, W_r, b_r):
    from concourse.bass_utils import run_bass_kernel_spmd

    f32 = np.float32
    bf16 = ml_dtypes.bfloat16
    u2 = np.ascontiguousarray(np.asarray(u, dtype=f32).reshape(S, KU))
    uh = _trunc11(u2)
    ul = (u2 - uh).astype(f32)
    uhT = np.ascontiguousarray(uh.T)
    ulT = np.ascontiguousarray(ul.T)
    u8T = np.ascontiguousarray(u2.T.astype(bf16))

    hx = np.concatenate([np.asarray(h, dtype=f32).ravel(),
                         np.asarray(us, dtype=f32).ravel(),
                         np.asarray(ue, dtype=f32).ravel()]).astype(f32)
    W_r = np.asarray(W_r, dtype=f32)
    wrT = np.ascontiguousarray(W_r.T)
    b_r = np.ascontiguousarray(np.asarray(b_r, dtype=f32))

    W_nn = np.asarray(W_nn, dtype=f32)
    W_no = np.asarray(W_no, dtype=f32)
    W_E = np.asarray(W_E, dtype=f32)
    b_nn = np.asarray(b_nn, dtype=f32)
    b_no = np.asarray(b_no, dtype=f32)
    b_E = np.asarray(b_E, dtype=f32)
    noise4 = np.asarray(noise, dtype=f32).reshape(S, DIM, NE)

    in_maps = []
    for c in range(NCORES):
        fsl = slice(c * FL, (c + 1) * FL)
        wnn_u = W_nn[fsl, :KU]
        wno_u = W_no[fsl, :KU]
        wE_u = W_E[fsl, :KU]
        wnn_h = _trunc11(wnn_u)
        wno_h = _trunc11(wno_u)
        wl8 = np.concatenate([(wnn_u - wnn_h).T.astype(bf16),
                              (wno_u - wno_h).T.astype(bf16)], axis=1)
        im = {
            "uhT": uhT, "ulT": ulT, "u8T": u8T,
            "whnnT": np.ascontiguousarray(wnn_h.T),
            "whnoT": np.ascontiguousarray(wno_h.T),
            "wl8T": np.ascontiguousarray(wl8),
            "we8T": np.ascontiguousarray(wE_u.T.astype(bf16)),
            "noise_c": np.ascontiguousarray(
                noise4[:, c * DL:(c + 1) * DL, :].reshape(S, FL)),
            "hxf": hx, "wrT": wrT, "b_r": b_r,
            "wRh_nn": np.ascontiguousarray(_trunc11(W_nn[fsl, KU:]).T),
            "wRl_nn": np.ascontiguousarray(
                (W_nn[fsl, KU:] - _trunc11(W_nn[fsl, KU:])).T.astype(f32)),
            "wRh_no": np.ascontiguousarray(_trunc11(W_no[fsl, KU:]).T),
            "wRl_no": np.ascontiguousarray(
                (W_no[fsl, KU:] - _trunc11(W_no[fsl, KU:])).T.astype(f32)),
            "wR_E": np.ascontiguousarray(W_E[fsl, KU:].T.astype(f32)),
            "bias_c": np.concatenate([b_nn[fsl], b_no[fsl], b_E[fsl]]).astype(f32),
        }
        in_maps.append(im)

    nc = _get_program()
    res = run_bass_kernel_spmd(nc, in_maps, core_ids=list(range(NCORES)),
                               trace=TRACE)
    _CACHE["last_results"] = res
    out = np.empty((1, S, DIM), dtype=f32)
    for c in range(NCORES):
        out[0, :, c * DL:(c + 1) * DL] = res.results[c]["out_c"]
    return out



# revision 6
# speedup vs baseline: 1.6673x; 1.6673x over previous
"""Trainium2 Bass kernel for nn_Experts (topk_masking).

Math (reference):
  R = concat(h,us,ue) @ W_r.T + b_r                       [1,1,512]
  x = concat(u, R.broadcast)                              [1,S,1536]
  h1 = (x @ W_nn.T + b_nn).reshape(S,512,16)
  h2 = (x @ W_no.T + b_no).reshape(S,512,16) * noise
  g  = top2-masked softmax over experts of (h1+h2)
  e  = (x @ W_E.T + b_E).reshape(S,512,16)
  out = (g*e).mean(-1)                                    [1,S,512]

Sharding: the NE*DIM output-feature dim of the three projections is sharded
across 8 cores (64 dims x 16 experts each, contiguous feature slice). The
token-independent R-path is folded into a per-feature constant c[f] computed
once per core, so the per-token matmuls contract only over u's 1024 features.

Precision: gating logits are computed as an fp16 10-bit-head matmul (PSUM
pre-scaled by 2^17 via the weight side) plus a single fp8 DoubleRow matmul
per k-chunk whose two K-slots carry the xl*w and x*wl correction terms at
the same 2^17 scale (fp8 products are exact in the PE's e10m10 path; only
the fp8 input rounding ~2^-4 of the ~2^-11 residuals matters, so logits land
at ~2^-15 accuracy). The e-matmul runs in bf16. Top-2 selection + softmax on
the scaled logits (max/compare are scale-invariant; exp applies 2^-17 via the
ACT engine's scale input).
"""
import numpy as np
import ml_dtypes

DIM = 512
NE = 16
S = 4096
KU = 2 * DIM        # u features = 1024
KR = DIM            # R features = 512
KX = 5 * DIM        # concat(h,us,ue) = 2560
NCORES = 8
DL = DIM // NCORES  # 64 dims per core
FL = DL * NE        # 1024 features per core
MCH = S // 128      # 32 token chunks
SCALE = np.float32(2.0 ** 17)
ISCALE = float(2.0 ** -17)

f16 = np.float16
bf16 = ml_dtypes.bfloat16
f8e4 = ml_dtypes.float8_e4m3   # TRN variant: max normal +-240

_MASK11 = np.uint32(0xFFFFF000)  # keep 11 explicit mantissa bits (truncate)

TRACE = False
_CACHE = {}


def _to_f8(a, scale):
    return np.clip(np.asarray(a, np.float32) * np.float32(scale),
                   -240.0, 240.0).astype(f8e4)


def _chunked(a):
    """[S, KU] -> [MCH, 128par(k%128), 8kc, 128tok] contiguous per chunk."""
    return np.ascontiguousarray(
        a.reshape(MCH, 128, 8, 128).transpose(0, 3, 2, 1))


def _build():
    import concourse.bass as bass
    import concourse.mybir as mybir
    import concourse.tile as tile
    from concourse import bacc
    from contextlib import ExitStack

    F32 = mybir.dt.float32
    F32R = mybir.dt.float32r
    F16 = mybir.dt.float16
    BF16 = mybir.dt.bfloat16
    F8 = mybir.dt.float8e4
    U32 = mybir.dt.uint32
    AX = mybir.AxisListType
    OP = mybir.AluOpType
    ACTF = mybir.ActivationFunctionType
    DR = mybir.MatmulPerfMode.DoubleRow

    nc = bacc.Bacc("TRN2", target_bir_lowering=False, debug=False,
                   num_devices=NCORES)

    def dram(name, shape, dt, kind="ExternalInput"):
        return nc.dram_tensor(name, shape, dt, kind=kind)

    # per-core inputs (same names on every core; data differs per core)
    xh16d = dram("xh16d", [MCH, 128, 8, 128], F16)       # fp16 head of u
    xc8d = dram("xc8d", [MCH, 128, 8, 2, 128], F8)       # fp8 (ul*2^11, u)
    x8d = dram("x8d", [MCH, 128, 8, 128], BF16)          # bf16 u (e-matmul)
    nzd = dram("nzd", [MCH, 128, FL], F32)               # noise slice
    wh1T = dram("wh1T", [KU, FL], F16)                   # fp16(W_nn*2^17).T
    wh2T = dram("wh2T", [KU, FL], F16)                   # fp16(W_no*2^17).T
    wc1T = dram("wc1T", [KU, 2, FL], F8)                 # (W_nn*2^6, wl*2^17).T
    wc2T = dram("wc2T", [KU, 2, FL], F8)
    we8T = dram("we8T", [KU, FL], BF16)
    hxf = dram("hxf", [KX], F32)
    wrT = dram("wrT", [KX, KR], F32)
    b_r = dram("b_r", [KR], F32)
    wRh_nn = dram("wRh_nn", [KR, FL], BF16)              # bf16 head of W[:,KU:]
    wRl_nn = dram("wRl_nn", [KR, FL], BF16)              # bf16 residual
    wRh_no = dram("wRh_no", [KR, FL], BF16)
    wRl_no = dram("wRl_no", [KR, FL], BF16)
    wRh_E = dram("wRh_E", [KR, FL], BF16)
    wRl_E = dram("wRl_E", [KR, FL], BF16)
    bias_c = dram("bias_c", [3 * FL], F32)
    out_c = dram("out_c", [S, DL], F32, kind="ExternalOutput")

    with tile.TileContext(nc) as tc, ExitStack() as ctx:
        wpool = ctx.enter_context(tc.tile_pool(name="w", bufs=1))

        # resident weights (one big DMA each); order = DMA priority
        wh2_t = wpool.tile([128, 8, FL], F16)
        nc.sync.dma_start(wh2_t[:], wh2T.ap().rearrange("(kc p) f -> p kc f", p=128))
        wh1_t = wpool.tile([128, 8, FL], F16)
        nc.sync.dma_start(wh1_t[:], wh1T.ap().rearrange("(kc p) f -> p kc f", p=128))
        wc2_t = wpool.tile([128, 8, 2, FL], F8)
        nc.sync.dma_start(wc2_t[:], wc2T.ap().rearrange("(kc p) two f -> p kc two f", p=128))
        wc1_t = wpool.tile([128, 8, 2, FL], F8)
        nc.sync.dma_start(wc1_t[:], wc1T.ap().rearrange("(kc p) two f -> p kc two f", p=128))
        we8_t = wpool.tile([128, 8, FL], BF16)
        nc.sync.dma_start(we8_t[:], we8T.ap().rearrange("(kc p) f -> p kc f", p=128))

        # survives the whole kernel: bias/R constant rows + ones for K=2 matmuls
        ccsb = wpool.tile([2, 3 * FL], F32R)
        onesf = wpool.tile([2, 128], F32)
        nc.vector.memset(onesf[:], 1.0)
        ones1 = wpool.tile([2, 128], F32R)
        nc.vector.tensor_copy(ones1[:], onesf[:])
        onessf = wpool.tile([2, 128], F32)
        nc.vector.memset(onessf[:], float(SCALE))
        onesS = wpool.tile([2, 128], F32R)
        nc.vector.tensor_copy(onesS[:], onessf[:])

        # ---------------- stage 0: R then c ----------------
        with ExitStack() as s0:
            s0sb = s0.enter_context(tc.tile_pool(name="s0sb", bufs=1))
            s0rot = s0.enter_context(tc.tile_pool(name="s0rot", bufs=4))
            s0ps = s0.enter_context(tc.tile_pool(name="s0ps", bufs=1, space="PSUM"))

            hx_t = s0sb.tile([128, 20], F32)
            nc.sync.dma_start(hx_t[:], hxf.ap().rearrange("(kc p) -> p kc", p=128))

            # R = hx @ W_r.T with W_r stationary: out lands as [128, 4]
            # across partitions directly (R[mo*128+p] = psR[p, mo]).
            psR = []
            for mo in range(4):
                psR_mo = s0ps.tile([128, 1], F32, tag=f"psR{mo}")
                psR.append(psR_mo)
            for kc in range(20):
                ksl = slice(kc * 128, (kc + 1) * 128)
                wr_ch = s0rot.tile([128, KR], F32, tag="rotf")
                nc.sync.dma_start(wr_ch[:], wrT.ap()[ksl, :])
                for mo in range(4):
                    msl = slice(mo * 128, (mo + 1) * 128)
                    nc.tensor.matmul(psR[mo][:], wr_ch[:, msl],
                                     hx_t[:, kc:kc + 1],
                                     start=(kc == 0), stop=(kc == 19))

            brt = s0sb.tile([128, 4], F32)
            nc.sync.dma_start(brt[:], b_r.ap().rearrange("(mo p) -> p mo", p=128))
            Rcol = s0sb.tile([128, 4], F32)
            for mo in range(4):
                nc.vector.tensor_add(Rcol[:, mo:mo + 1], psR[mo][:],
                                     brt[:, mo:mo + 1])

            # bf16 head + bf16 residual of R for the c-path matmuls
            Rh8 = s0sb.tile([128, 4], BF16)
            nc.vector.tensor_copy(Rh8[:], Rcol[:])
            Rhf = s0sb.tile([128, 4], F32)
            nc.vector.tensor_copy(Rhf[:], Rh8[:])
            Rlf = s0sb.tile([128, 4], F32)
            nc.vector.tensor_sub(Rlf[:], Rcol[:], Rhf[:])
            Rbh = s0sb.tile([128, 4, 128], BF16)
            nc.vector.tensor_copy(Rbh[:], Rh8[:].broadcast_to([128, 4, 128]))
            Rbl = s0sb.tile([128, 4, 128], BF16)
            nc.vector.tensor_copy(Rbl[:], Rlf[:].broadcast_to([128, 4, 128]))

            # c pieces: 0 -> c_nn, 1 -> c_no, 2 -> c_E (each FL wide)
            # c = Rh*Wh + Rl*Wh + Rh*Wl  (all bf16 operands)
            biasb2 = s0sb.tile([1, 3 * FL], F32)
            nc.sync.dma_start(biasb2[:],
                              bias_c.ap().rearrange("(o f) -> o f", o=1))
            cpsum = s0ps.tile([128, FL], F32, tag="cps")
            pieces = [(wRh_nn, wRl_nn), (wRh_no, wRl_no), (wRh_E, wRl_E)]
            for pi, (wh_d, wl_d) in enumerate(pieces):
                for kc in range(4):
                    ksl = slice(kc * 128, (kc + 1) * 128)
                    for half in range(2):
                        fsl = slice(half * 512, (half + 1) * 512)
                        whch = s0rot.tile([128, 512], BF16, tag="rot")
                        nc.sync.dma_start(whch[:], wh_d.ap()[ksl, fsl])
                        wlch = s0rot.tile([128, 512], BF16, tag="rot2")
                        nc.sync.dma_start(wlch[:], wl_d.ap()[ksl, fsl])
                        nc.tensor.matmul(cpsum[:, fsl], Rbh[:, kc, :], whch[:],
                                         start=(kc == 0), stop=False)
                        nc.tensor.matmul(cpsum[:, fsl], Rbl[:, kc, :], whch[:],
                                         start=False, stop=False)
                        nc.tensor.matmul(cpsum[:, fsl], Rbh[:, kc, :], wlch[:],
                                         start=False, stop=(kc == 3))
                psl = slice(pi * FL, (pi + 1) * FL)
                nc.vector.tensor_add(biasb2[0:1, psl], cpsum[0:1, :],
                                     biasb2[0:1, psl])

            # split c into 11-bit head + residual on partition 0, round both
            # to f32r, then DMA into the two rows of ccsb
            cht = s0sb.tile([1, 3 * FL], F32)
            nc.vector.tensor_scalar(cht[0:1, :].bitcast(U32),
                                    biasb2[0:1, :].bitcast(U32),
                                    int(_MASK11), None, OP.bitwise_and)
            clt = s0sb.tile([1, 3 * FL], F32)
            nc.vector.tensor_sub(clt[0:1, :], biasb2[0:1, :], cht[0:1, :])
            chr_ = s0sb.tile([1, 3 * FL], F32R)
            nc.vector.tensor_copy(chr_[0:1, :], cht[0:1, :])
            clr_ = s0sb.tile([1, 3 * FL], F32R)
            nc.vector.tensor_copy(clr_[0:1, :], clt[0:1, :])
            nc.sync.dma_start(ccsb[0:1, :], chr_[0:1, :])
            nc.sync.dma_start(ccsb[1:2, :], clr_[0:1, :])

        # ---------------- main loop over 32 token chunks ----------------
        spool = ctx.enter_context(tc.tile_pool(name="stream", bufs=2))
        epool = ctx.enter_context(tc.tile_pool(name="epi", bufs=2))
        mpsum = ctx.enter_context(tc.tile_pool(name="mps", bufs=1, space="PSUM"))

        for m in range(MCH):
            tsl = slice(m * 128, (m + 1) * 128)
            xh_t = spool.tile([128, 8, 128], F16, tag="xh")
            xc_t = spool.tile([128, 8, 2, 128], F8, tag="xc")
            x8_t = spool.tile([128, 8, 128], BF16, tag="x8")
            nz_t = spool.tile([128, FL], F32, tag="nz")
            nc.sync.dma_start(xh_t[:], xh16d.ap()[m])
            nc.sync.dma_start(xc_t[:], xc8d.ap()[m])
            nc.sync.dma_start(x8_t[:], x8d.ap()[m])
            nc.sync.dma_start(nz_t[:], nzd.ap()[m])

            h1p = mpsum.tile([128, FL], F32, tag="h1")
            h2p = mpsum.tile([128, FL], F32, tag="h2")
            ep = mpsum.tile([128, FL], F32, tag="e")

            # gating blocks: fp16-head mains + fp8 DoubleRow corrections +
            # scaled bias/R-constant, all accumulated at 2^17 scale
            for psum_t, wh_t, wc_t, coff in ((h2p, wh2_t, wc2_t, FL),
                                             (h1p, wh1_t, wc1_t, 0)):
                for k in range(8):
                    st = (k == 0)
                    for half in range(2):
                        fsl = slice(half * 512, (half + 1) * 512)
                        nc.tensor.matmul(psum_t[:, fsl], xh_t[:, k, :],
                                         wh_t[:, k, fsl], start=st, stop=False)
                for k in range(8):
                    for half in range(2):
                        fsl = slice(half * 512, (half + 1) * 512)
                        nc.tensor.matmul(psum_t[:, fsl], xc_t[:, k, :, :],
                                         wc_t[:, k, :, fsl], start=False,
                                         stop=False, perf_mode=DR)
                for half in range(2):
                    fsl = slice(half * 512, (half + 1) * 512)
                    csl = slice(coff + half * 512, coff + (half + 1) * 512)
                    nc.tensor.matmul(psum_t[:, fsl], onesS[:], ccsb[:, csl],
                                     start=False, stop=(half == 1))

            # e block (bf16, unscaled)
            for k in range(8):
                st = (k == 0)
                for half in range(2):
                    fsl = slice(half * 512, (half + 1) * 512)
                    nc.tensor.matmul(ep[:, fsl], x8_t[:, k, :],
                                     we8_t[:, k, fsl], start=st, stop=False)
            for half in range(2):
                fsl = slice(half * 512, (half + 1) * 512)
                csl = slice(2 * FL + half * 512, 2 * FL + (half + 1) * 512)
                nc.tensor.matmul(ep[:, fsl], ones1[:], ccsb[:, csl],
                                 start=False, stop=(half == 1))

            # ---------------- epilogue (scaled logits) ----------------
            t_t = epool.tile([128, FL], F32, tag="t")
            nc.vector.tensor_mul(t_t[:], h2p[:], nz_t[:])
            m_t = epool.tile([128, FL], F32, tag="m")
            nc.vector.tensor_add(m_t[:], t_t[:], h1p[:])

            mg = m_t[:].rearrange("p (d e) -> p d e", e=NE)
            v1 = epool.tile([128, DL], F32, tag="v1")
            nc.vector.tensor_reduce(v1[:], mg, AX.X, op=OP.max)
            eq1 = epool.tile([128, FL], F32, tag="eq1")
            nc.vector.tensor_tensor(eq1[:].rearrange("p (d e) -> p d e", e=NE),
                                    mg, v1[:].broadcast_to([128, DL, NE]),
                                    OP.is_equal)
            m2 = epool.tile([128, FL], F32, tag="m2")
            nc.vector.scalar_tensor_tensor(m2[:], eq1[:], -1e30, m_t[:],
                                           OP.mult, OP.add)
            v2 = epool.tile([128, DL], F32, tag="v2")
            nc.vector.tensor_reduce(v2[:], m2[:].rearrange("p (d e) -> p d e", e=NE),
                                    AX.X, op=OP.max)
            minv = epool.tile([128, FL], F32, tag="minv")
            nc.vector.tensor_tensor(minv[:].rearrange("p (d e) -> p d e", e=NE),
                                    mg, v2[:].broadcast_to([128, DL, NE]),
                                    OP.is_lt)
            mmsk = epool.tile([128, FL], F32, tag="mmsk")
            nc.vector.scalar_tensor_tensor(mmsk[:], minv[:], -1e30, m_t[:],
                                           OP.mult, OP.add)
            q8 = epool.tile([128, FL], BF16, tag="q8")
            nc.scalar.activation(q8[:], mmsk[:], ACTF.Exp, scale=ISCALE)
            e8 = epool.tile([128, FL], BF16, tag="e8")
            nc.scalar.activation(e8[:], ep[:], ACTF.Copy)
            t2 = epool.tile([128, FL], BF16, tag="t2")
            nc.vector.tensor_mul(t2[:], q8[:], e8[:])
            s_t = epool.tile([128, DL], F32, tag="s")
            nc.vector.tensor_reduce(s_t[:], t2[:].rearrange("p (d e) -> p d e", e=NE),
                                    AX.X, op=OP.add)

            ev12 = epool.tile([128, 2 * DL], F32, tag="ev12")
            nc.scalar.activation(ev12[:, :DL], v1[:], ACTF.Exp, scale=ISCALE)
            nc.scalar.activation(ev12[:, DL:], v2[:], ACTF.Exp, scale=ISCALE)
            z_t = epool.tile([128, DL], F32, tag="z")
            nc.vector.tensor_add(z_t[:], ev12[:, :DL], ev12[:, DL:])
            r_t = epool.tile([128, DL], F32, tag="r")
            nc.vector.reciprocal(r_t[:], z_t[:])
            o_t = epool.tile([128, DL], F32, tag="o")
            nc.vector.scalar_tensor_tensor(o_t[:], s_t[:], 1.0 / NE, r_t[:],
                                           OP.mult, OP.mult)
            nc.sync.dma_start(out_c.ap()[tsl, :], o_t[:])

    nc.compile()
    return nc


def _get_program():
    if "nc" not in _CACHE:
        _CACHE["nc"] = _build()
    return _CACHE["nc"]


def kernel(h, us, ue, u, noise, W_nn, b_nn, W_no, b_no, W_E, b_E, W_r, b_r):
    from concourse.bass_utils import run_bass_kernel_spmd

    f32 = np.float32
    u2 = np.ascontiguousarray(np.asarray(u, dtype=f32).reshape(S, KU))
    uh16 = u2.astype(f16)
    ul = (u2 - uh16.astype(f32)).astype(f32)

    xh16c = _chunked(uh16)
    x8c = _chunked(u2.astype(bf16))
    xl8s = _chunked(_to_f8(ul, 2.0 ** 11))
    xf8s = _chunked(_to_f8(u2, 1.0))
    xc8c = np.ascontiguousarray(np.stack([xl8s, xf8s], axis=3))

    hx = np.concatenate([np.asarray(h, dtype=f32).ravel(),
                         np.asarray(us, dtype=f32).ravel(),
                         np.asarray(ue, dtype=f32).ravel()]).astype(f32)
    W_r = np.asarray(W_r, dtype=f32)
    wrT = np.ascontiguousarray(W_r.T)
    b_r = np.ascontiguousarray(np.asarray(b_r, dtype=f32))

    W_nn = np.asarray(W_nn, dtype=f32)
    W_no = np.asarray(W_no, dtype=f32)
    W_E = np.asarray(W_E, dtype=f32)
    b_nn = np.asarray(b_nn, dtype=f32)
    b_no = np.asarray(b_no, dtype=f32)
    b_E = np.asarray(b_E, dtype=f32)
    noise4 = np.asarray(noise, dtype=f32).reshape(S, DIM, NE)

    in_maps = []
    for c in range(NCORES):
        fsl = slice(c * FL, (c + 1) * FL)

        def gate_parts(W):
            Wu = W[fsl, :KU]
            wh16 = (Wu * SCALE).astype(f16)        # 10-bit head at 2^17 scale
            wl = (Wu - wh16.astype(f32) / SCALE).astype(f32)
            whT = np.ascontiguousarray(wh16.T)     # [KU, FL] fp16
            wcT = np.ascontiguousarray(np.stack(
                [_to_f8(Wu, 2.0 ** 6).T, _to_f8(wl, 2.0 ** 17).T], axis=1))
            return whT, wcT

        wh1T, wc1T = gate_parts(W_nn)
        wh2T, wc2T = gate_parts(W_no)

        def cparts(W):
            WR = W[fsl, KU:]
            wh = WR.astype(bf16)
            wl = (WR - wh.astype(f32)).astype(bf16)
            return (np.ascontiguousarray(wh.T), np.ascontiguousarray(wl.T))

        wRh_nn, wRl_nn = cparts(W_nn)
        wRh_no, wRl_no = cparts(W_no)
        wRh_E, wRl_E = cparts(W_E)

        im = {
            "xh16d": xh16c, "xc8d": xc8c, "x8d": x8c,
            "nzd": np.ascontiguousarray(
                noise4[:, c * DL:(c + 1) * DL, :].reshape(MCH, 128, FL)),
            "wh1T": wh1T, "wh2T": wh2T, "wc1T": wc1T, "wc2T": wc2T,
            "we8T": np.ascontiguousarray(W_E[fsl, :KU].T.astype(bf16)),
            "hxf": hx, "wrT": wrT, "b_r": b_r,
            "wRh_nn": wRh_nn, "wRl_nn": wRl_nn,
            "wRh_no": wRh_no, "wRl_no": wRl_no,
            "wRh_E": wRh_E, "wRl_E": wRl_E,
            "bias_c": np.concatenate([b_nn[fsl], b_no[fsl], b_E[fsl]]).astype(f32),
        }
        in_maps.append(im)

    nc = _get_program()
    res = run_bass_kernel_spmd(nc, in_maps, core_ids=list(range(NCORES)),
                               trace=TRACE)
    _CACHE["last_results"] = res
    out = np.empty((1, S, DIM), dtype=f32)
    for c in range(NCORES):
        out[0, :, c * DL:(c + 1) * DL] = res.results[c]["out_c"]
    return out


# revision 8
# speedup vs baseline: 1.8280x; 1.0964x over previous
"""Trainium2 Bass kernel for nn_Experts (topk_masking).

Math (reference):
  R = concat(h,us,ue) @ W_r.T + b_r                       [1,1,512]
  x = concat(u, R.broadcast)                              [1,S,1536]
  h1 = (x @ W_nn.T + b_nn).reshape(S,512,16)
  h2 = (x @ W_no.T + b_no).reshape(S,512,16) * noise
  g  = top2-masked softmax over experts of (h1+h2)
  e  = (x @ W_E.T + b_E).reshape(S,512,16)
  out = (g*e).mean(-1)                                    [1,S,512]

Sharding: the NE*DIM output-feature dim of the three projections is sharded
across 8 cores (64 dims x 16 experts each, contiguous feature slice). The
token-independent R-path is folded into a per-feature constant c[f] computed
once per core, so the per-token matmuls contract only over u's 1024 features.

Precision: gating logits are computed as an fp16 10-bit-head matmul (PSUM
pre-scaled by 2^17 via the weight side) plus a single fp8 DoubleRow matmul
per k-chunk whose two K-slots carry the xl*w and x*wl correction terms at
the same 2^17 scale (fp8 products are exact in the PE's e10m10 path; only
the fp8 input rounding ~2^-4 of the ~2^-11 residuals matters, so logits land
at ~2^-15 accuracy). The e-matmul runs in bf16. The R matvec and the R-path
constants use the same fp16-head + fp8-residual trick. Top-2 selection +
softmax run on the scaled logits (max/compare are scale-invariant; exp
applies 2^-17 via the ACT engine's scale input).
"""
import numpy as np
import ml_dtypes

DIM = 512
NE = 16
S = 4096
KU = 2 * DIM        # u features = 1024
KR = DIM            # R features = 512
KX = 5 * DIM        # concat(h,us,ue) = 2560
NCORES = 8
DL = DIM // NCORES  # 64 dims per core
FL = DL * NE        # 1024 features per core
MCH = S // 128      # 32 token chunks
SCALE = np.float32(2.0 ** 17)
ISCALE = float(2.0 ** -17)

f16 = np.float16
bf16 = ml_dtypes.bfloat16
f8e4 = ml_dtypes.float8_e4m3   # TRN variant: max normal +-240

_MASK11 = np.uint32(0xFFFFF000)  # keep 11 explicit mantissa bits (truncate)

TRACE = False
_CACHE = {}


def _to_f8(a, scale):
    return np.clip(np.asarray(a, np.float32) * np.float32(scale),
                   -240.0, 240.0).astype(f8e4)


def _chunked(a):
    """[S, KU] -> [MCH, 128par(k%128), 8kc, 128tok] contiguous per chunk."""
    return np.ascontiguousarray(
        a.reshape(MCH, 128, 8, 128).transpose(0, 3, 2, 1))


def _build():
    import concourse.bass as bass
    import concourse.mybir as mybir
    import concourse.tile as tile
    from concourse import bacc
    from contextlib import ExitStack

    F32 = mybir.dt.float32
    F32R = mybir.dt.float32r
    F16 = mybir.dt.float16
    BF16 = mybir.dt.bfloat16
    F8 = mybir.dt.float8e4
    U32 = mybir.dt.uint32
    AX = mybir.AxisListType
    OP = mybir.AluOpType
    ACTF = mybir.ActivationFunctionType
    DR = mybir.MatmulPerfMode.DoubleRow

    nc = bacc.Bacc("TRN2", target_bir_lowering=False, debug=False,
                   num_devices=NCORES)

    def dram(name, shape, dt, kind="ExternalInput"):
        return nc.dram_tensor(name, shape, dt, kind=kind)

    # per-core inputs (same names on every core; data differs per core)
    xh16d = dram("xh16d", [MCH, 128, 8, 128], F16)       # fp16 head of u
    xc8d = dram("xc8d", [MCH, 128, 8, 2, 128], F8)       # fp8 (ul*2^11, u)
    x8d = dram("x8d", [MCH, 128, 8, 128], BF16)          # bf16 u (e-matmul)
    nzd = dram("nzd", [MCH, 128, FL], F32)               # noise slice
    wh1T = dram("wh1T", [KU, FL], F16)                   # fp16(W_nn*2^17).T
    wh2T = dram("wh2T", [KU, FL], F16)                   # fp16(W_no*2^17).T
    wl1T = dram("wl1T", [KU, FL], F8)                    # fp8(resid_nn*2^17).T
    wl2T = dram("wl2T", [KU, FL], F8)
    we8T = dram("we8T", [KU, FL], BF16)
    hxh = dram("hxh", [KX], F16)                         # fp16 head of hx
    hxl = dram("hxl", [KX], F16)                         # fp16 residual
    hx8 = dram("hx8", [KX], F8)                          # fp8 of hx
    wr16T = dram("wr16T", [KX, KR], F16)                 # fp16(W_r*2^17).T
    wrl8T = dram("wrl8T", [KX, KR], F8)                  # fp8(resid*2^17).T
    b_r = dram("b_r", [KR], F32)
    wc16_nn = dram("wc16_nn", [KR, FL], F16)             # fp16(W[:,KU:]*2^17).T
    wcl8_nn = dram("wcl8_nn", [KR, FL], F8)              # fp8(resid*2^17).T
    wc16_no = dram("wc16_no", [KR, FL], F16)
    wcl8_no = dram("wcl8_no", [KR, FL], F8)
    wE8 = dram("wE8", [KR, FL], BF16)                    # bf16 W_E[:,KU:].T
    bias_c = dram("bias_c", [3 * FL], F32)
    out_c = dram("out_c", [S, DL], F32, kind="ExternalOutput")

    with tile.TileContext(nc) as tc, ExitStack() as ctx:
        wpool = ctx.enter_context(tc.tile_pool(name="w", bufs=1))
        spool = ctx.enter_context(tc.tile_pool(name="stream", bufs=2))

        # ---- DMA issue order == DMA priority (single queue) ----
        # 1. stage-0 inputs (R matvec + c-path weights)
        with ExitStack() as s0:
            s0sb = s0.enter_context(tc.tile_pool(name="s0sb", bufs=1))
            s0ps = s0.enter_context(tc.tile_pool(name="s0ps", bufs=1, space="PSUM"))

            hxh_t = s0sb.tile([128, 20], F16)
            nc.sync.dma_start(hxh_t[:], hxh.ap().rearrange("(kc p) -> p kc", p=128))
            hxl_t = s0sb.tile([128, 20], F16)
            nc.sync.dma_start(hxl_t[:], hxl.ap().rearrange("(kc p) -> p kc", p=128))
            hx8_t = s0sb.tile([128, 20], F8)
            nc.sync.dma_start(hx8_t[:], hx8.ap().rearrange("(kc p) -> p kc", p=128))
            brt = s0sb.tile([128, 4], F32)
            nc.sync.dma_start(brt[:], b_r.ap().rearrange("(mo p) -> p mo", p=128))
            biasb2 = s0sb.tile([1, 3 * FL], F32)
            nc.sync.dma_start(biasb2[:],
                              bias_c.ap().rearrange("(o f) -> o f", o=1))
            wr16_t = s0sb.tile([128, 20, KR], F16)
            nc.sync.dma_start(wr16_t[:],
                              wr16T.ap().rearrange("(kc p) m -> p kc m", p=128))
            wrl8_t = s0sb.tile([128, 20, KR], F8)
            nc.sync.dma_start(wrl8_t[:],
                              wrl8T.ap().rearrange("(kc p) m -> p kc m", p=128))
            cw_nn = s0sb.tile([128, 4, FL], F16)
            nc.sync.dma_start(cw_nn[:],
                              wc16_nn.ap().rearrange("(kc p) f -> p kc f", p=128))
            cl_nn = s0sb.tile([128, 4, FL], F8)
            nc.sync.dma_start(cl_nn[:],
                              wcl8_nn.ap().rearrange("(kc p) f -> p kc f", p=128))
            cw_no = s0sb.tile([128, 4, FL], F16)
            nc.sync.dma_start(cw_no[:],
                              wc16_no.ap().rearrange("(kc p) f -> p kc f", p=128))
            cl_no = s0sb.tile([128, 4, FL], F8)
            nc.sync.dma_start(cl_no[:],
                              wcl8_no.ap().rearrange("(kc p) f -> p kc f", p=128))
            cwE = s0sb.tile([128, 4, FL], BF16)
            nc.sync.dma_start(cwE[:],
                              wE8.ap().rearrange("(kc p) f -> p kc f", p=128))

            # 2. chunk-0 streams (ahead of the big resident weights)
            xh_t0 = spool.tile([128, 8, 128], F16, tag="xh")
            nc.sync.dma_start(xh_t0[:], xh16d.ap()[0])
            xc_t0 = spool.tile([128, 8, 2, 128], F8, tag="xc")
            nc.sync.dma_start(xc_t0[:], xc8d.ap()[0])
            x8_t0 = spool.tile([128, 8, 128], BF16, tag="x8")
            nc.sync.dma_start(x8_t0[:], x8d.ap()[0])
            nz_t0 = spool.tile([128, FL], F32, tag="nz")
            nc.sync.dma_start(nz_t0[:], nzd.ap()[0])

            # 3. resident weights, in per-chunk consumption order
            wh2_t = wpool.tile([128, 8, FL], F16)
            nc.sync.dma_start(wh2_t[:], wh2T.ap().rearrange("(kc p) f -> p kc f", p=128))
            wc2_t = wpool.tile([128, 8, 2, FL], F8)
            nc.sync.dma_start(wc2_t[:, :, 1, :],
                              wl2T.ap().rearrange("(kc p) f -> p kc f", p=128))
            wh1_t = wpool.tile([128, 8, FL], F16)
            nc.sync.dma_start(wh1_t[:], wh1T.ap().rearrange("(kc p) f -> p kc f", p=128))
            wc1_t = wpool.tile([128, 8, 2, FL], F8)
            nc.sync.dma_start(wc1_t[:, :, 1, :],
                              wl1T.ap().rearrange("(kc p) f -> p kc f", p=128))
            we8_t = wpool.tile([128, 8, FL], BF16)
            nc.sync.dma_start(we8_t[:], we8T.ap().rearrange("(kc p) f -> p kc f", p=128))

            # corr slot 0 (fp8(W*2^6)) derived on-device from the fp16 heads
            nc.vector.tensor_scalar(wc2_t[:, :, 0, :], wh2_t[:],
                                    float(2.0 ** -11), None, OP.mult)
            nc.vector.tensor_scalar(wc1_t[:, :, 0, :], wh1_t[:],
                                    float(2.0 ** -11), None, OP.mult)

            # constants: ones rows for the K=2 bias matmuls (f32 storage,
            # bitcast to f32r at the matmul)
            ccsb = wpool.tile([2, 3 * FL], F32)
            onesf = wpool.tile([2, 128], F32)
            nc.vector.memset(onesf[:], 1.0)
            onessf = wpool.tile([2, 128], F32)
            nc.vector.memset(onessf[:], float(SCALE))

            # ---------------- stage 0 compute: R then c ----------------
            # R matvec at 2^17 scale: (hxh+hxl fp16)*wr16 + hx8*wrl8
            psR = []
            for mo in range(4):
                psR_mo = s0ps.tile([128, 1], F32, tag=f"psR{mo}")
                psR.append(psR_mo)
            for kc in range(20):
                for mo in range(4):
                    msl = slice(mo * 128, (mo + 1) * 128)
                    nc.tensor.matmul(psR[mo][:], wr16_t[:, kc, msl],
                                     hxh_t[:, kc:kc + 1],
                                     start=(kc == 0), stop=False)
                    nc.tensor.matmul(psR[mo][:], wr16_t[:, kc, msl],
                                     hxl_t[:, kc:kc + 1],
                                     start=False, stop=False)
                    nc.tensor.matmul(psR[mo][:], wrl8_t[:, kc, msl],
                                     hx8_t[:, kc:kc + 1],
                                     start=False, stop=(kc == 19))

            Rcol = s0sb.tile([128, 4], F32)
            for mo in range(4):
                nc.vector.scalar_tensor_tensor(Rcol[:, mo:mo + 1], psR[mo][:],
                                               ISCALE, brt[:, mo:mo + 1],
                                               OP.mult, OP.add)

            # fp16 head + fp16 residual + fp8 of R, broadcast along tokens
            Rh16 = s0sb.tile([128, 4], F16)
            nc.vector.tensor_copy(Rh16[:], Rcol[:])
            Rhf = s0sb.tile([128, 4], F32)
            nc.vector.tensor_copy(Rhf[:], Rh16[:])
            Rlf = s0sb.tile([128, 4], F32)
            nc.vector.tensor_sub(Rlf[:], Rcol[:], Rhf[:])
            Rl16 = s0sb.tile([128, 4], F16)
            nc.vector.tensor_copy(Rl16[:], Rlf[:])
            R8c = s0sb.tile([128, 4], F8)
            nc.vector.tensor_copy(R8c[:], Rcol[:])
            Rbh = s0sb.tile([128, 4, 128], F16)
            nc.vector.tensor_copy(Rbh[:], Rh16[:].broadcast_to([128, 4, 128]))
            Rbl = s0sb.tile([128, 4, 128], F16)
            nc.vector.tensor_copy(Rbl[:], Rl16[:].broadcast_to([128, 4, 128]))
            Rb8 = s0sb.tile([128, 4, 128], F8)
            nc.vector.tensor_copy(Rb8[:], R8c[:].broadcast_to([128, 4, 128]))

            # c pieces: 0 -> c_nn, 1 -> c_no (2^17-scaled), 2 -> c_E (unscaled)
            cpsum = s0ps.tile([128, FL], F32, tag="cps")
            for pi, (cw_t, cl_t) in enumerate(((cw_nn, cl_nn), (cw_no, cl_no))):
                for kc in range(4):
                    for half in range(2):
                        fsl = slice(half * 512, (half + 1) * 512)
                        nc.tensor.matmul(cpsum[:, fsl], Rbh[:, kc, :],
                                         cw_t[:, kc, fsl],
                                         start=(kc == 0), stop=False)
                        nc.tensor.matmul(cpsum[:, fsl], Rbl[:, kc, :],
                                         cw_t[:, kc, fsl],
                                         start=False, stop=False)
                        nc.tensor.matmul(cpsum[:, fsl], Rb8[:, kc, :],
                                         cl_t[:, kc, fsl],
                                         start=False, stop=(kc == 3))
                psl = slice(pi * FL, (pi + 1) * FL)
                nc.vector.scalar_tensor_tensor(biasb2[0:1, psl], cpsum[0:1, :],
                                               ISCALE, biasb2[0:1, psl],
                                               OP.mult, OP.add)
            for kc in range(4):
                for half in range(2):
                    fsl = slice(half * 512, (half + 1) * 512)
                    nc.tensor.matmul(cpsum[:, fsl], Rbh[:, kc, :],
                                     cwE[:, kc, fsl], start=(kc == 0), stop=False)
                    nc.tensor.matmul(cpsum[:, fsl], Rbl[:, kc, :],
                                     cwE[:, kc, fsl], start=False, stop=(kc == 3))
            nc.vector.tensor_add(biasb2[0:1, 2 * FL:], cpsum[0:1, :],
                                 biasb2[0:1, 2 * FL:])

            # split c into 11-bit head + residual on partition 0, round both
            # to f32r, then DMA into the two rows of ccsb
            cht = s0sb.tile([1, 3 * FL], F32)
            nc.vector.tensor_scalar(cht[0:1, :].bitcast(U32),
                                    biasb2[0:1, :].bitcast(U32),
                                    int(_MASK11), None, OP.bitwise_and)
            nc.vector.tensor_sub(biasb2[0:1, :], biasb2[0:1, :], cht[0:1, :])
            nc.sync.dma_start(ccsb[0:1, :], cht[0:1, :])
            nc.sync.dma_start(ccsb[1:2, :], biasb2[0:1, :])

        # ---------------- main loop over 32 token chunks ----------------
        epool = ctx.enter_context(tc.tile_pool(name="epi", bufs=2))
        mpsum = ctx.enter_context(tc.tile_pool(name="mps", bufs=1, space="PSUM"))
        for m in range(MCH):
            tsl = slice(m * 128, (m + 1) * 128)
            if m == 0:
                xh_t, xc_t, x8_t, nz_t = xh_t0, xc_t0, x8_t0, nz_t0
            else:
                xh_t = spool.tile([128, 8, 128], F16, tag="xh")
                xc_t = spool.tile([128, 8, 2, 128], F8, tag="xc")
                x8_t = spool.tile([128, 8, 128], BF16, tag="x8")
                nz_t = spool.tile([128, FL], F32, tag="nz")
                nc.sync.dma_start(xh_t[:], xh16d.ap()[m])
                nc.sync.dma_start(xc_t[:], xc8d.ap()[m])
                nc.sync.dma_start(x8_t[:], x8d.ap()[m])
                nc.sync.dma_start(nz_t[:], nzd.ap()[m])

            h1p = mpsum.tile([128, FL], F32, tag="h1")
            h2p = mpsum.tile([128, FL], F32, tag="h2")
            ep = mpsum.tile([128, FL], F32, tag="e")

            # gating blocks: fp16-head mains + fp8 DoubleRow corrections +
            # scaled bias/R-constant, all accumulated at 2^17 scale
            for psum_t, wh_t, wc_t, coff in ((h2p, wh2_t, wc2_t, FL),
                                             (h1p, wh1_t, wc1_t, 0)):
                for k in range(8):
                    st = (k == 0)
                    for half in range(2):
                        fsl = slice(half * 512, (half + 1) * 512)
                        nc.tensor.matmul(psum_t[:, fsl], xh_t[:, k, :],
                                         wh_t[:, k, fsl], start=st, stop=False)
                for k in range(8):
                    for half in range(2):
                        fsl = slice(half * 512, (half + 1) * 512)
                        nc.tensor.matmul(psum_t[:, fsl], xc_t[:, k, :, :],
                                         wc_t[:, k, :, fsl], start=False,
                                         stop=False, perf_mode=DR)
                for half in range(2):
                    fsl = slice(half * 512, (half + 1) * 512)
                    csl = slice(coff + half * 512, coff + (half + 1) * 512)
                    nc.tensor.matmul(psum_t[:, fsl],
                                     onessf[:].bitcast(F32R),
                                     ccsb[:, csl].bitcast(F32R),
                                     start=False, stop=(half == 1))

            # e block (bf16, unscaled)
            for k in range(8):
                st = (k == 0)
                for half in range(2):
                    fsl = slice(half * 512, (half + 1) * 512)
                    nc.tensor.matmul(ep[:, fsl], x8_t[:, k, :],
                                     we8_t[:, k, fsl], start=st, stop=False)
            for half in range(2):
                fsl = slice(half * 512, (half + 1) * 512)
                csl = slice(2 * FL + half * 512, 2 * FL + (half + 1) * 512)
                nc.tensor.matmul(ep[:, fsl], onesf[:].bitcast(F32R),
                                 ccsb[:, csl].bitcast(F32R),
                                 start=False, stop=(half == 1))

            # ---------------- epilogue (scaled logits) ----------------
            t_t = epool.tile([128, FL], F32, tag="t")
            nc.vector.tensor_mul(t_t[:], h2p[:], nz_t[:])
            m_t = epool.tile([128, FL], F32, tag="m")
            nc.vector.tensor_add(m_t[:], t_t[:], h1p[:])

            mg = m_t[:].rearrange("p (d e) -> p d e", e=NE)
            v1 = epool.tile([128, DL], F32, tag="v1")
            nc.vector.tensor_reduce(v1[:], mg, AX.X, op=OP.max)
            eq1 = epool.tile([128, FL], F32, tag="eq1")
            nc.vector.tensor_tensor(eq1[:].rearrange("p (d e) -> p d e", e=NE),
                                    mg, v1[:].broadcast_to([128, DL, NE]),
                                    OP.is_equal)
            m2 = epool.tile([128, FL], F32, tag="m2")
            nc.vector.scalar_tensor_tensor(m2[:], eq1[:], -1e30, m_t[:],
                                           OP.mult, OP.add)
            v2 = epool.tile([128, DL], F32, tag="v2")
            nc.vector.tensor_reduce(v2[:], m2[:].rearrange("p (d e) -> p d e", e=NE),
                                    AX.X, op=OP.max)
            minv = epool.tile([128, FL], F32, tag="minv")
            nc.vector.tensor_tensor(minv[:].rearrange("p (d e) -> p d e", e=NE),
                                    mg, v2[:].broadcast_to([128, DL, NE]),
                                    OP.is_lt)
            mmsk = epool.tile([128, FL], F32, tag="mmsk")
            nc.vector.scalar_tensor_tensor(mmsk[:], minv[:], -1e30, m_t[:],
                                           OP.mult, OP.add)
            q8 = epool.tile([128, FL], BF16, tag="q8")
            nc.scalar.activation(q8[:], mmsk[:], ACTF.Exp, scale=ISCALE)
            e8 = epool.tile([128, FL], BF16, tag="e8")
            nc.scalar.activation(e8[:], ep[:], ACTF.Copy)
            t2 = epool.tile([128, FL], BF16, tag="t2")
            nc.vector.tensor_mul(t2[:], q8[:], e8[:])
            s_t = epool.tile([128, DL], F32, tag="s")
            nc.vector.tensor_reduce(s_t[:], t2[:].rearrange("p (d e) -> p d e", e=NE),
                                    AX.X, op=OP.add)

            ev12 = epool.tile([128, 2 * DL], F32, tag="ev12")
            nc.scalar.activation(ev12[:, :DL], v1[:], ACTF.Exp, scale=ISCALE)
            nc.scalar.activation(ev12[:, DL:], v2[:], ACTF.Exp, scale=ISCALE)
            z_t = epool.tile([128, DL], F32, tag="z")
            nc.vector.tensor_add(z_t[:], ev12[:, :DL], ev12[:, DL:])
            r_t = epool.tile([128, DL], F32, tag="r")
            nc.vector.reciprocal(r_t[:], z_t[:])
            o_t = epool.tile([128, DL], F32, tag="o")
            nc.vector.scalar_tensor_tensor(o_t[:], s_t[:], 1.0 / NE, r_t[:],
                                           OP.mult, OP.mult)
            nc.sync.dma_start(out_c.ap()[tsl, :], o_t[:])

    nc.compile()
    return nc


def _get_program():
    if "nc" not in _CACHE:
        _CACHE["nc"] = _build()
    return _CACHE["nc"]


def kernel(h, us, ue, u, noise, W_nn, b_nn, W_no, b_no, W_E, b_E, W_r, b_r):
    from concourse.bass_utils import run_bass_kernel_spmd

    f32 = np.float32
    u2 = np.ascontiguousarray(np.asarray(u, dtype=f32).reshape(S, KU))
    uh16 = u2.astype(f16)
    ul = (u2 - uh16.astype(f32)).astype(f32)

    xh16c = _chunked(uh16)
    x8c = _chunked(u2.astype(bf16))
    xl8s = _chunked(_to_f8(ul, 2.0 ** 11))
    xf8s = _chunked(_to_f8(u2, 1.0))
    xc8c = np.ascontiguousarray(np.stack([xl8s, xf8s], axis=3))

    hx = np.concatenate([np.asarray(h, dtype=f32).ravel(),
                         np.asarray(us, dtype=f32).ravel(),
                         np.asarray(ue, dtype=f32).ravel()]).astype(f32)
    hxh = hx.astype(f16)
    hxl = (hx - hxh.astype(f32)).astype(f16)
    hx8 = _to_f8(hx, 1.0)
    W_r = np.asarray(W_r, dtype=f32)
    wr16 = (W_r * SCALE).astype(f16)                    # [KR, KX]
    wrl8 = _to_f8(W_r - wr16.astype(f32) / SCALE, 2.0 ** 17)
    b_r = np.ascontiguousarray(np.asarray(b_r, dtype=f32))

    W_nn = np.asarray(W_nn, dtype=f32)
    W_no = np.asarray(W_no, dtype=f32)
    W_E = np.asarray(W_E, dtype=f32)
    b_nn = np.asarray(b_nn, dtype=f32)
    b_no = np.asarray(b_no, dtype=f32)
    b_E = np.asarray(b_E, dtype=f32)
    noise4 = np.asarray(noise, dtype=f32).reshape(S, DIM, NE)

    in_maps = []
    for c in range(NCORES):
        fsl = slice(c * FL, (c + 1) * FL)

        def head_resid(Wblk):
            wh = (Wblk * SCALE).astype(f16)
            wl8 = _to_f8(Wblk - wh.astype(f32) / SCALE, 2.0 ** 17)
            return (np.ascontiguousarray(wh.T), np.ascontiguousarray(wl8.T))

        wh1T, wl1T = head_resid(W_nn[fsl, :KU])
        wh2T, wl2T = head_resid(W_no[fsl, :KU])
        wc16_nn, wcl8_nn = head_resid(W_nn[fsl, KU:])
        wc16_no, wcl8_no = head_resid(W_no[fsl, KU:])

        im = {
            "xh16d": xh16c, "xc8d": xc8c, "x8d": x8c,
            "nzd": np.ascontiguousarray(
                noise4[:, c * DL:(c + 1) * DL, :].reshape(MCH, 128, FL)),
            "wh1T": wh1T, "wh2T": wh2T, "wl1T": wl1T, "wl2T": wl2T,
            "we8T": np.ascontiguousarray(W_E[fsl, :KU].T.astype(bf16)),
            "hxh": hxh, "hxl": hxl, "hx8": hx8,
            "wr16T": np.ascontiguousarray(wr16.T),
            "wrl8T": np.ascontiguousarray(wrl8.T),
            "b_r": b_r,
            "wc16_nn": wc16_nn, "wcl8_nn": wcl8_nn,
            "wc16_no": wc16_no, "wcl8_no": wcl8_no,
            "wE8": np.ascontiguousarray(W_E[fsl, KU:].T.astype(bf16)),
            "bias_c": np.concatenate([b_nn[fsl], b_no[fsl], b_E[fsl]]).astype(f32),
        }
        in_maps.append(im)

    nc = _get_program()
    res = run_bass_kernel_spmd(nc, in_maps, core_ids=list(range(NCORES)),
                               trace=TRACE)
    _CACHE["last_results"] = res
    out = np.empty((1, S, DIM), dtype=f32)
    for c in range(NCORES):
        out[0, :, c * DL:(c + 1) * DL] = res.results[c]["out_c"]
    return out
